# revision 10
# baseline (speedup 1.0000x reference)
"""Linear (kernel-feature-map) attention on Trainium2 via Bass/Tile.

Shapes: B,H,S,D = 4,16,4096,64.  B*H = 64 independent head-problems,
sharded 8 per NeuronCore across 8 cores (pure head parallelism).

Math per head (identical to the reference up to fp32 rounding; the
reference normalizes q first, row scaling commutes with the matmul):
    ksum[d]  = sum_s K[s,d]
    denom[s] = Q[s,:] . ksum (+eps, negligible vs denom ~6e4)
    KV[d,e]  = sum_s K[s,d] V[s,e]
    out[s,e] = (Q[s,:] @ KV[:,e]) / denom[s]

Heads are processed in PAIRS packed into the 128-wide PE array.  The host
repacks inputs into one array qkv[pair, t, 128, 385] whose columns are
[K_A|K_B | V_A|V_B | ones | Q_A|Q_B] per 128-row s-tile, so each s-tile is
ONE contiguous 192KB DMA and every matmul carries at most ONE sync wait
(this toolchain's LDWEIGHTS encoding has a single wait slot).

Per pair:
  mm1:  lhsT=[K_A|K_B] [s128,128], rhs=[V_A|V_B|ones] [s128,129] -> PSUM
        [128,129] accumulated over 32 s-tiles: diagonal blocks KV_A/KV_B,
        col 128 = [ksum_A;ksum_B].  Off-diagonal garbage ignored.
  qT:   PE transpose of [Q_A|Q_B] tiles -> [d128,s128], DVE-copied to SBUF.
  mm2:  lhsT=qT tile, rhs=[blockdiag(KV_A,KV_B)|ksumA;0|0;ksumB] [128,130]
        -> PSUM [s128,130]: cols 0:128 unnormalized out, 128:130 denoms.
  DVE:  rcp = 1/denom; out = unnorm * rcp ([p,1] broadcast); DMA out.

All PSUM consumers live on the vector engine so every mm2's RAW+WAR deps
coalesce into one DVE wait.
"""

import sys
import numpy as np

try:
    import concourse.bass as bass  # noqa: F401
except ImportError:  # fresh grading dir: repo is normally on sys.path via site
    for p in ("/opt/trn_rl_repo", "/root/.axon_site/_ro/trn_rl_repo"):
        if p not in sys.path:
            sys.path.insert(0, p)
    import concourse.bass as bass  # noqa: F401

B, H, S, D = 4, 16, 4096, 64
NCORES = 8
HPC = (B * H) // NCORES      # 8 heads per core
NPAIR = HPC // 2             # 4 head-pairs per core
NT = S // 128                # 32 s-tiles of 128 rows
QKV_W = 385                  # K(128) | V(128) | ones(1) | Q(128)


def _build_nc():
    import concourse.bass as bass
    import concourse.tile as tile
    from concourse import mybir
    from concourse.masks import make_identity

    f32 = mybir.dt.float32
    nc = bass.Bass(num_swdge_queues=4)
    qkvp = nc.declare_dram_parameter("qkv", [NPAIR, NT, 128, QKV_W], f32,
                                     isOutput=False)
    op = nc.declare_dram_parameter("o", [HPC, S, D], f32, isOutput=True)

    with tile.TileContext(nc) as tc:
        with (
            tc.tile_pool(name="const", bufs=1) as const_pool,
            tc.tile_pool(name="qkvin", bufs=16) as in_pool,
            tc.tile_pool(name="qt", bufs=2) as qt_pool,
            tc.tile_pool(name="outbuf", bufs=4) as out_pool,
            tc.tile_pool(name="small", bufs=4) as small_pool,
            tc.tile_pool(name="ps_kv", bufs=2, space="PSUM") as ps_kv_pool,
            tc.tile_pool(name="ps_qt", bufs=4, space="PSUM") as ps_qt_pool,
            tc.tile_pool(name="ps_out", bufs=2, space="PSUM") as ps_out_pool,
        ):
            ident = const_pool.tile([128, 128], f32)
            make_identity(nc, ident)
            # PE gate: absorb the Pool-sem dep once so later matmuls don't.
            ps_warm = ps_qt_pool.tile([128, 128], f32, tag="psqt")
            nc.tensor.transpose(ps_warm, ident, ident)

            for pr in range(NPAIR):
                hA = 2 * pr
                od = op[hA:hA + 2].rearrange("h (t p) d -> p t h d", p=128)

                qt_all = qt_pool.tile([128, S], f32, tag="qt")
                obig = out_pool.tile([128, S], f32, tag="obig")
                ps_kv = ps_kv_pool.tile([128, 129], f32, tag="pskv")
                for t in range(NT):
                    qkv_t = in_pool.tile([128, QKV_W], f32, tag="qkv")
                    nc.sync.dma_start(out=qkv_t, in_=qkvp[pr, t])
                    nc.tensor.matmul(
                        ps_kv,
                        lhsT=qkv_t[:, 0:128],
                        rhs=qkv_t[:, 128:257],
                        start=(t == 0),
                        stop=(t == NT - 1),
                        skip_group_check=True,
                    )
                    ps_qt = ps_qt_pool.tile([128, 128], f32, tag="psqt")
                    nc.tensor.transpose(ps_qt, qkv_t[:, 257:385], ident)
                    nc.vector.tensor_copy(
                        out=qt_all[:, t * 128:(t + 1) * 128], in_=ps_qt
                    )

                rhs2 = small_pool.tile([128, 130], f32, tag="rhs2")
                nc.vector.memset(rhs2, 0.0)
                nc.vector.tensor_copy(out=rhs2[0:64, 0:64], in_=ps_kv[0:64, 0:64])
                nc.vector.tensor_copy(
                    out=rhs2[64:128, 64:128], in_=ps_kv[64:128, 64:128]
                )
                nc.vector.tensor_copy(
                    out=rhs2[0:64, 128:129], in_=ps_kv[0:64, 128:129]
                )
                nc.vector.tensor_copy(
                    out=rhs2[64:128, 129:130], in_=ps_kv[64:128, 128:129]
                )

                for t in range(NT):
                    ps_o = ps_out_pool.tile([128, 130], f32, tag="pso")
                    nc.tensor.matmul(
                        ps_o,
                        lhsT=qt_all[:, t * 128:(t + 1) * 128],
                        rhs=rhs2,
                        start=True,
                        stop=True,
                    )
                    rcp = small_pool.tile([128, 2], f32, tag="rcp")
                    nc.vector.reciprocal(rcp, ps_o[:, 128:130])
                    ob = obig[:, t * 128:(t + 1) * 128]
                    nc.vector.tensor_scalar_mul(
                        out=ob[:, 0:64], in0=ps_o[:, 0:64],
                        scalar1=rcp[:, 0:1],
                    )
                    nc.vector.tensor_scalar_mul(
                        out=ob[:, 64:128], in0=ps_o[:, 64:128],
                        scalar1=rcp[:, 1:2],
                    )
                    nc.gpsimd.dma_start(
                        out=od[:, t],
                        in_=ob.rearrange('p (h d) -> p h d', h=2),
                    )
    return nc


def _legalize_waits(nc):
    """Split multi-wait instructions into single-wait NoOps + instruction.

    This toolchain's walrus codegen accepts at most ONE sync wait per
    instruction ("Too many sync wait commands").  Engines execute their
    stream in order, so hoisting all-but-one wait onto preceding NoOps on
    the same engine is semantically identical.
    """
    import concourse.mybir as mybir

    for f in nc.m.functions:
        for blk in f.blocks:
            il = blk.instructions
            if not any(
                i.sync_info is not None and len(i.sync_info.on_wait) > 1
                for i in il
            ):
                continue
            new = []
            for inst in il:
                si = inst.sync_info
                if si is not None and len(si.on_wait) > 1:
                    waits = list(si.on_wait)
                    for j, w in enumerate(waits[:-1]):
                        new.append(mybir.InstNoOp(
                            name=f"{inst.name}-lw{j}",
                            engine=inst.engine,
                            sync_info=mybir.SyncInfo(on_wait=[w], on_update=[]),
                        ))
                    inst.sync_info = mybir.SyncInfo(
                        on_wait=[waits[-1]], on_update=list(si.on_update)
                    )
                new.append(inst)
            blk.instructions = new


_NC_CACHE = None


def _get_nc():
    global _NC_CACHE
    if _NC_CACHE is None:
        nc = _build_nc()
        _legalize_waits(nc)
        _NC_CACHE = nc
    return _NC_CACHE


def _pack(x):
    # [HPC, S, D] -> [NPAIR, NT, 128, 2*D] with columns [head_A | head_B]
    return np.ascontiguousarray(
        x.reshape(NPAIR, 2, NT, 128, D).transpose(0, 2, 3, 1, 4)
        .reshape(NPAIR, NT, 128, 2 * D)
    )


def _make_in_maps(query_layer, key_layer, value_layer):
    q = np.asarray(query_layer, dtype=np.float32).reshape(B * H, S, D)
    k = np.asarray(key_layer, dtype=np.float32).reshape(B * H, S, D)
    v = np.asarray(value_layer, dtype=np.float32).reshape(B * H, S, D)
    maps = []
    ones = np.ones((NPAIR, NT, 128, 1), dtype=np.float32)
    for c in range(NCORES):
        sl = slice(c * HPC, (c + 1) * HPC)
        qkv = np.concatenate(
            [_pack(k[sl]), _pack(v[sl]), ones, _pack(q[sl])], axis=-1
        )
        maps.append({"qkv": np.ascontiguousarray(qkv)})
    return maps


def kernel(query_layer, key_layer, value_layer):
    from concourse.bass_utils import run_bass_kernel_spmd

    nc = _get_nc()
    in_maps = _make_in_maps(query_layer, key_layer, value_layer)
    res = run_bass_kernel_spmd(nc, in_maps, list(range(NCORES)))
    out = np.stack([res.results[c]["o"] for c in range(NCORES)], axis=0)
    return out.reshape(B, H, S, D).astype(np.float32)


def run_profiled(inputs, trace_cores=None):
    """test.py helper: run with NTFF tracing, return BassKernelResults."""
    from concourse.bass_utils import run_bass_kernel_spmd

    nc = _get_nc()
    in_maps = _make_in_maps(**inputs)
    return run_bass_kernel_spmd(
        nc, in_maps, list(range(NCORES)), trace=True,
        trace_cores=trace_cores,
    )
